# revision 37
# baseline (speedup 1.0000x reference)
"""MI-LSTM (attention LSTM) + LSTM + linear head for Trainium2, 8-core batch-parallel.

Model (per timestep, per batch row b):
  gm = y@W_main + h@U_main + b_main -> i,f,o,cm gates
  ga[k] = x_k@W_aux[k] + h@U_aux[k] + b_aux[k] -> i_k (sigmoid), c_k (tanh)
  candidates l = [i*cm, i_k*c_k] (9, H)
  u_k = tanh(l_k . (W_att @ c) + b_att); a = softmax(u); L = sum a_k l_k
  c' = f*c + L; h' = o*tanh(c')
Then a standard LSTM over the h-sequence, then relu + linear to scalar.

Mapping: batch sharded 8 ways (256 rows/core = 2 partition tiles of 128).
Phase-1 attention math is batch-major. x/y inputs are pre-transposed to
feature-major on the HOST (with a ones row for biases). Weight columns are
grouped [sig-gates(704) | tanh-gates(576)] per tau so each tau needs one
sigmoid + one tanh activation; the two tau streams are split through
matmul->activation->l->z->first tree level so they overlap, rejoining at
the softmax. Phase 2 (standard LSTM, feature-major, gates in column
blocks) runs one step behind, its matmuls+activations emitted into
phase-1's softmax window. exp() via (1+t)/(1-t), t=tanh(u/2).
"""

import os
import numpy as np
import ml_dtypes

import concourse.bacc as bacc
import concourse.bass as bass
import concourse.mybir as mybir
from concourse.tile import TileContext, add_dep_helper
from concourse.bass_utils import run_bass_kernel_spmd

F32 = mybir.dt.float32
BF16 = mybir.dt.bfloat16
ALU = mybir.AluOpType
ACTF = mybir.ActivationFunctionType
AX = mybir.AxisListType

S, B, F, H, K = 256, 2048, 5, 64, 8
NC = 8
BL = B // NC          # 256 batch rows per core
NT = BL // 128        # 2 partition tiles
NCAND = K + 1         # 9 candidates
XR_ROWS = 46          # 5 y + 40 x + 1 ones (bias row)
CH = 8                # steps per x-slab DMA chunk
SIGW = 704            # sig-gate block: i_main, i_aux(512), f, o
TANW = 576            # tanh-gate block: cm, c_aux(512)
GW = SIGW + TANW      # 1280 gate columns per tau

LAST_RESULTS = {}


def _bank_splits(s, e, cuts=()):
    """Split [s, e) at 512-col PSUM bank boundaries and explicit cut points."""
    out = []
    while s < e:
        nxt = min(e, (s // 512 + 1) * 512)
        for c in cuts:
            if s < c < nxt:
                nxt = c
        out.append((s, nxt))
        s = nxt
    return out


def _build(n_steps: int, b_att: float):
    nc = bacc.Bacc(None, target_bir_lowering=False)

    xin = nc.dram_tensor("xin", [n_steps, XR_ROWS, BL], BF16, kind="ExternalInput")
    w46 = nc.dram_tensor("w46", [XR_ROWS, GW], BF16, kind="ExternalInput")
    wh = nc.dram_tensor("wh", [H, GW], BF16, kind="ExternalInput")
    watt = nc.dram_tensor("watt", [H, H], BF16, kind="ExternalInput")
    wca2 = nc.dram_tensor("wca2", [H + 1, 4 * H], BF16, kind="ExternalInput")
    wcb = nc.dram_tensor("wcb", [H, 4 * H], BF16, kind="ExternalInput")
    linw = nc.dram_tensor("linw", [H, 1], BF16, kind="ExternalInput")
    idf32 = nc.dram_tensor("idf32", [128, 128], F32, kind="ExternalInput")
    onesrow = nc.dram_tensor("onesrow", [1, 8 * BL], BF16, kind="ExternalInput")
    out = nc.dram_tensor("out", [n_steps, BL, 1], F32, kind="ExternalOutput")

    with TileContext(nc) as tc:
        with (
            tc.tile_pool(name="state", bufs=1) as st,
            tc.tile_pool(name="wts", bufs=1) as wp,
            tc.tile_pool(name="work", bufs=2) as wk,
            tc.tile_pool(name="xr", bufs=2) as xrp,
            tc.tile_pool(name="gpsum", bufs=1, space="PSUM") as gp,
            tc.tile_pool(name="mpsum", bufs=1, space="PSUM") as mp,
            tc.tile_pool(name="p2psum", bufs=1, space="PSUM") as p2p,
        ):
            W46 = wp.tile([XR_ROWS, GW], BF16, tag="w46")
            WH = wp.tile([H, GW], BF16, tag="wh")
            WA = wp.tile([H, H], BF16, tag="watt")
            WCA2 = wp.tile([H + 1, 4 * H], BF16, tag="wca2")
            WCB = wp.tile([H, 4 * H], BF16, tag="wcb")
            LW = wp.tile([H, 1], BF16, tag="linw")
            IDF = wp.tile([128, 128], F32, tag="idf32")
            for t_, d_ in ((W46, w46), (WH, wh), (WA, watt), (WCA2, wca2),
                           (WCB, wcb), (LW, linw), (IDF, idf32)):
                nc.sync.dma_start(t_[:], d_[:])

            HC1 = st.tile([128, 2 * 128], F32, tag="hc1")    # [h|c] per tau
            HST = st.tile([H + 1, 8 * BL], BF16, tag="hst")  # h1^T ring + ones
            OUTS = st.tile([H + 1, 8 * BL], F32, tag="outs")
            H2T = st.tile([H, BL], BF16, tag="h2t")          # phase-2 h^T
            C2 = st.tile([H, BL], F32, tag="c2")             # phase-2 c

            nc.vector.memset(HC1[:], 0.0)
            nc.vector.memset(H2T[:], 0.0)
            nc.vector.memset(C2[:], 0.0)
            nc.sync.dma_start(HST[H:H + 1, :], onesrow[:])   # bias row

            xch = xin.rearrange("(c n) r b -> c r n b", n=CH)
            n_chunks = (n_steps + CH - 1) // CH
            xr_tiles = {}
            for c in range(min(2, n_chunks)):
                xr_tiles[c] = xrp.tile([XR_ROWS, CH * BL], BF16, tag="xr",
                                       name="xrt%d" % c)
                nc.sync.dma_start(
                    xr_tiles[c][:].rearrange("r (n b) -> r n b", n=CH), xch[c])

            outv = out.rearrange("(c s) b one -> c one (s b)", s=min(8, n_steps))

            def ph2_mm_act(u, after_mm=None, after_act=None):
                """Phase-2 step u front half: gate matmuls + activations.
                g2h gate column blocks: [i|f|o (sigmoid) | g (tanh)].
                Ordering hints keep these off phase-1's critical path."""
                sl2 = (u + 1) % 8
                hs = HST[:, sl2 * BL:(sl2 + 1) * BL]
                g2h = p2p.tile([128, 4 * BL], F32, tag="g2h")
                for p_ in range(4):
                    o0 = p_ * BL
                    mi = nc.tensor.matmul(g2h[0:H, o0:o0 + BL],
                                          WCA2[:, p_ * H:(p_ + 1) * H],
                                          hs, start=True, stop=False)
                    if p_ == 0 and after_mm is not None:
                        add_dep_helper(after_mm.ins, mi.ins, sync=False,
                                       reason="ph2 mms after ph1 gate mms")
                    del mi
                    nc.tensor.matmul(g2h[0:H, o0:o0 + BL],
                                     WCB[:, p_ * H:(p_ + 1) * H],
                                     H2T[:], start=False, stop=True)
                sfo2 = wk.tile([H, 3 * BL], BF16, tag="sfo2")   # [i|f|o]
                g2t = wk.tile([H, BL], BF16, tag="g2t")
                a1 = nc.scalar.activation(sfo2[:], g2h[0:H, 0:3 * BL], ACTF.Sigmoid)
                if after_act is not None:
                    add_dep_helper(after_act.ins, a1.ins, sync=True,
                                   reason="ph2 acts in the aw/Lp window")
                nc.scalar.activation(g2t[:], g2h[0:H, 3 * BL:4 * BL], ACTF.Tanh)
                return g2h, sfo2, g2t

            def ph2_tail(u, g2h, sfo2, g2t):
                """Phase-2 step u back half: state update + head."""
                ig = wk.tile([H, BL], BF16, tag="ig")
                nc.vector.tensor_mul(ig[:], sfo2[:, 0:BL], g2t[:])
                fc2 = wk.tile([H, BL], F32, tag="fc2")
                nc.vector.tensor_mul(fc2[:], sfo2[:, BL:2 * BL], C2[:])
                nc.vector.tensor_tensor(C2[:], ig[:], fc2[:], ALU.add)
                tc2 = wk.tile([H, BL], BF16, tag="tc2")
                nc.scalar.activation(tc2[:], C2[:], ACTF.Tanh)
                nc.vector.tensor_mul(H2T[:], sfo2[:, 2 * BL:3 * BL], tc2[:])
                rh = wk.tile([H, BL], BF16, tag="rh")
                nc.vector.tensor_scalar_max(rh[:], H2T[:], 0.0)
                nc.tensor.matmul(g2h[H:H + 1, 0:BL], LW[:], rh[:],
                                 start=True, stop=True)
                ob = u % 8
                nc.scalar.activation(OUTS[H:H + 1, ob * BL:(ob + 1) * BL],
                                     g2h[H:H + 1, 0:BL], ACTF.Copy)
                if ob == 7 or u == n_steps - 1:
                    nc.sync.dma_start(outv[u // 8], OUTS[H:H + 1, 0:(ob + 1) * BL])

            ph2_pend = None
            ph2_front = None
            prev_t2 = None
            PH2_LAG = 4

            # ================= main loop =================
            for t in range(n_steps):
                # -- h/c transposes -> misc cols 0:256; v-mm result 256:384
                misc = mp.tile([128, 512], F32, tag="misc")
                for tau in range(NT):
                    nc.tensor.transpose(
                        misc[0:128, tau * 128:(tau + 1) * 128],
                        HC1[:, tau * 128:(tau + 1) * 128], IDF[:])
                nc.scalar.activation(HST[0:H, (t % 8) * BL:(t % 8 + 1) * BL],
                                     misc[0:H, 0:256], ACTF.Copy)
                CT = wk.tile([H, BL], BF16, tag="ct")
                nc.scalar.activation(CT[:], misc[H:128, 0:256], ACTF.Copy)

                # -- gate matmuls, tau0 fully first so its ACTs start early
                ch, sl = t // CH, t % CH
                XR = xr_tiles[ch]
                if sl == 0 and ch + 1 < n_chunks and (ch + 1) not in xr_tiles:
                    xr_tiles[ch + 1] = xrp.tile([XR_ROWS, CH * BL], BF16,
                                                tag="xr", name="xrt%d" % (ch + 1))
                    nc.sync.dma_start(
                        xr_tiles[ch + 1][:].rearrange("r (n b) -> r n b", n=CH),
                        xch[ch + 1])
                xr_tiles.pop(ch - 1, None)
                hsl = HST[0:H, (t % 8) * BL:(t % 8 + 1) * BL]
                gps = gp.tile([128, NT * GW], F32, tag="gates")
                sigfo = wk.tile([128, NT * SIGW], BF16, tag="sigfo")
                tan = wk.tile([128, NT * TANW], BF16, tag="tan")
                l_t = wk.tile([128, NT * 576], BF16, tag="l")
                vS = wk.tile([128, NT * H], BF16, tag="vs")
                for tau in range(NT):
                    xl = XR[:, sl * BL + tau * 128:sl * BL + (tau + 1) * 128]
                    hl = hsl[:, tau * 128:(tau + 1) * 128]
                    for (s_, e_) in _bank_splits(tau * GW, (tau + 1) * GW,
                                                 cuts=(tau * GW + SIGW,)):
                        w0, w1 = s_ - tau * GW, e_ - tau * GW
                        nc.tensor.matmul(gps[:, s_:e_], xl, W46[:, w0:w1],
                                         start=True, stop=False)
                        mm_i = nc.tensor.matmul(gps[:, s_:e_], hl,
                                                WH[:, w0:w1],
                                                start=False, stop=True)
                    if tau == 0:
                        # v = (W_att @ c)^T while tau1 matmuls run
                        for t2_ in range(NT):
                            nc.tensor.matmul(
                                misc[:, 256 + t2_ * H:256 + (t2_ + 1) * H],
                                CT[:, t2_ * 128:(t2_ + 1) * 128], WA[:],
                                start=True, stop=True)
                    # critical sigmoid (i-gates, 576) + tanh (576), then l;
                    nc.scalar.activation(sigfo[:, tau * SIGW:(tau + 1) * SIGW],
                                         gps[:, tau * GW:tau * GW + SIGW],
                                         ACTF.Sigmoid)
                    tan_i = nc.scalar.activation(
                        tan[:, tau * TANW:(tau + 1) * TANW],
                        gps[:, tau * GW + SIGW:(tau + 1) * GW], ACTF.Tanh)
                    nc.vector.tensor_mul(l_t[:, tau * 576:(tau + 1) * 576],
                                         sigfo[:, tau * SIGW:tau * SIGW + 576],
                                         tan[:, tau * TANW:(tau + 1) * TANW])
                nc.scalar.activation(vS[:], misc[:, 256:384], ACTF.Copy)

                # z = l * v (broadcast over candidates); first u-tree level
                z_t = wk.tile([128, NT * 576], BF16, tag="z")
                zt1 = wk.tile([128, NT * NCAND * 32], BF16, tag="zt1")
                for tau in range(NT):
                    lv = l_t[:, tau * 576:(tau + 1) * 576].rearrange(
                        "p (k h) -> p k h", k=NCAND)
                    vb = (vS[:, tau * H:(tau + 1) * H].unsqueeze(1)
                          .broadcast_to((128, NCAND, H)))
                    zv = z_t[:, tau * 576:(tau + 1) * 576].rearrange(
                        "p (k h) -> p k h", k=NCAND)
                    nc.vector.tensor_tensor(zv, lv, vb, ALU.mult)
                    z1v = zt1[:, tau * 288:(tau + 1) * 288].rearrange(
                        "p (k h) -> p k h", k=NCAND)
                    nc.vector.tensor_tensor(z1v, zv[:, :, 0:32], zv[:, :, 32:64],
                                            ALU.add)

                # joint u-tree tail: 32 -> 16 -> 8 -> reduce
                z1j = zt1[:].rearrange("p (t k h) -> p t k h", k=NCAND, h=32)
                zt2 = wk.tile([128, NT * NCAND * 16], BF16, tag="zt2")
                z2j = zt2[:].rearrange("p (t k h) -> p t k h", k=NCAND, h=16)
                nc.vector.tensor_tensor(z2j, z1j[:, :, :, 0:16],
                                        z1j[:, :, :, 16:32], ALU.add)
                zt3 = wk.tile([128, NT * NCAND * 8], BF16, tag="zt3")
                z3j = zt3[:].rearrange("p (t k h) -> p t k h", k=NCAND, h=8)
                nc.vector.tensor_tensor(z3j, z2j[:, :, :, 0:8],
                                        z2j[:, :, :, 8:16], ALU.add)
                u_t = wk.tile([128, NT * NCAND], F32, tag="u")
                nc.vector.tensor_reduce(
                    u_t[:].rearrange("p (t k) -> p t k", k=NCAND), z3j,
                    AX.X, ALU.add)

                # softmax via exp(u) = (1+t2)/(1-t2), t2 = tanh(u/2)
                ut2 = wk.tile([128, NT * NCAND], F32, tag="ut2")
                nc.scalar.activation(ut2[:], u_t[:], ACTF.Tanh, bias=b_att,
                                     scale=1.0)
                t2 = wk.tile([128, NT * NCAND], F32, tag="t2")
                t2_i = nc.scalar.activation(t2[:], ut2[:], ACTF.Tanh, scale=0.5)
                q_t = wk.tile([128, NT * NCAND], F32, tag="q")
                nc.vector.tensor_scalar(q_t[:], t2[:], -1.0, 1.0, ALU.mult, ALU.add)
                rq = wk.tile([128, NT * NCAND], F32, tag="rq")
                nc.vector.reciprocal_approx_fast(rq[:], q_t[:])
                rp = wk.tile([128, NT * NCAND * 2], BF16, tag="rp")
                rpv = rp[:].rearrange("p (c two) -> p c two", two=2)
                nc.vector.scalar_tensor_tensor(
                    rpv[:, :, 0:1], t2[:].unsqueeze(2), 1.0, rq[:].unsqueeze(2),
                    ALU.add, ALU.mult)
                nc.vector.tensor_copy(rpv[:, :, 1:2], rpv[:, :, 0:1])
                s_t = wk.tile([128, NT], F32, tag="s")
                nc.vector.tensor_reduce(
                    s_t[:],
                    rp[:].rearrange("p (t k two) -> p t two k", t=2,
                                    two=2)[:, :, 0:1, :],
                    AX.X, ALU.add)
                rs = wk.tile([128, NT], F32, tag="rs")
                nc.vector.reciprocal_approx_fast(rs[:], s_t[:])

                # phase-2 front half, pinned into the aw/Lp-tree window of
                # a slightly earlier step via the scheduler's manual wait
                if ph2_pend is not None:
                    with tc.tile_wait_until(0.030 + (ph2_pend + 2) * 0.0125):
                        ph2_front = ph2_mm_act(ph2_pend, after_mm=mm_i,
                                               after_act=prev_t2)

                # aw = l * exp(u); Lp-tree reduces candidates (9 = 8+1)
                aw = wk.tile([128, NT * 576], BF16, tag="aw")
                lp4 = l_t[:].rearrange("p (c h2 two) -> p c h2 two", h2=32, two=2)
                rb4 = (rp[:].rearrange("p (c two) -> p c two", two=2)
                       .unsqueeze(2).broadcast_to((128, NT * NCAND, 32, 2)))
                nc.vector.tensor_tensor(
                    aw[:].rearrange("p (c h2 two) -> p c h2 two", h2=32, two=2),
                    lp4, rb4, ALU.mult)
                awv = aw[:].rearrange("p (t k h) -> p t k h", k=NCAND, h=H)
                at1 = wk.tile([128, NT * 4 * H], BF16, tag="at1")
                a1v = at1[:].rearrange("p (t k h) -> p t k h", k=4, h=H)
                nc.vector.tensor_tensor(a1v, awv[:, :, 0:4, :],
                                        awv[:, :, 4:8, :], ALU.add)
                at2 = wk.tile([128, NT * 2 * H], BF16, tag="at2")
                a2v = at2[:].rearrange("p (t k h) -> p t k h", k=2, h=H)
                nc.vector.tensor_tensor(a2v, a1v[:, :, 0:2, :],
                                        a1v[:, :, 2:4, :], ALU.add)
                at3 = wk.tile([128, NT * H], F32, tag="at3")
                nc.vector.tensor_tensor(
                    at3[:].rearrange("p (t h) -> p t h", h=H).unsqueeze(2),
                    a2v[:, :, 0:1, :], a2v[:, :, 1:2, :], ALU.add)
                Lp = wk.tile([128, NT * H], F32, tag="Lp")
                nc.vector.tensor_tensor(
                    Lp[:].rearrange("p (t h) -> p t h", h=H).unsqueeze(2),
                    at3[:].rearrange("p (t h) -> p t h", h=H).unsqueeze(2),
                    awv[:, :, 8:9, :], ALU.add)

                prev_t2 = t2_i

                # state update: c' = f*c + rs*Lp; h' = o*tanh(c')
                hc1v = HC1[:].rearrange("p (t x) -> p t x", t=2)
                sfv = sigfo[:].rearrange("p (t c) -> p t c", t=2)
                fc = wk.tile([128, NT * H], F32, tag="fc")
                nc.vector.tensor_tensor(
                    fc[:].rearrange("p (t h) -> p t h", t=2),
                    sfv[:, :, 576:640], hc1v[:, :, 64:128], ALU.mult)
                for tau in range(NT):
                    nc.vector.scalar_tensor_tensor(
                        HC1[:, tau * 128 + 64:tau * 128 + 128],
                        Lp[:, tau * H:(tau + 1) * H], rs[:, tau:tau + 1],
                        fc[:, tau * H:(tau + 1) * H], ALU.mult, ALU.add)
                tc1 = wk.tile([128, NT * H], BF16, tag="tc1")
                nc.scalar.activation(
                    tc1[:].rearrange("p (t h) -> p t h", t=2),
                    hc1v[:, :, 64:128], ACTF.Tanh)
                nc.vector.tensor_tensor(
                    hc1v[:, :, 0:64],
                    sfv[:, :, 640:704],
                    tc1[:].rearrange("p (t h) -> p t h", t=2), ALU.mult)

                # phase-2 back half
                if ph2_pend is not None:
                    ph2_tail(ph2_pend, *ph2_front)
                if t >= PH2_LAG - 1:
                    ph2_pend = t - (PH2_LAG - 1)

            # epilogue: h1(n-1) -> ring slot, then last phase-2 steps
            misc = mp.tile([128, 512], F32, tag="misc")
            for tau in range(NT):
                nc.tensor.transpose(
                    misc[0:128, tau * 128:(tau + 1) * 128],
                    HC1[:, tau * 128:(tau + 1) * 128], IDF[:])
            nc.scalar.activation(
                HST[0:H, (n_steps % 8) * BL:(n_steps % 8 + 1) * BL],
                misc[0:H, 0:256], ACTF.Copy)
            for u in range(max(0, n_steps - PH2_LAG), n_steps):
                f_ = ph2_mm_act(u)
                ph2_tail(u, *f_)

    nc.finalize()
    return nc


def _prep_weights(inp):
    f32 = np.float32
    W_main, U_main, b_main = (np.asarray(inp["W_main"], f32),
                              np.asarray(inp["U_main"], f32),
                              np.asarray(inp["b_main"], f32))
    W_aux, U_aux, b_aux = (np.asarray(inp["W_aux"], f32),
                           np.asarray(inp["U_aux"], f32),
                           np.asarray(inp["b_aux"], f32))
    # column layout: [i_main | i_aux(8x64) | f | o || cm | c_aux(8x64)]
    w46 = np.zeros((XR_ROWS, GW), f32)
    wh = np.zeros((H, GW), f32)
    w46[0:5, 0:64] = W_main[:, 0:64]          # i_main
    w46[45, 0:64] = b_main[0:64]
    wh[:, 0:64] = U_main[:, 0:64]
    for k in range(K):
        c = 64 + 64 * k
        w46[5 + 5 * k:10 + 5 * k, c:c + 64] = W_aux[k, :, 0:64]
        w46[45, c:c + 64] = b_aux[k, 0:64]
        wh[:, c:c + 64] = U_aux[k, :, 0:64]
    w46[0:5, 576:640] = W_main[:, 64:128]     # f
    w46[45, 576:640] = b_main[64:128]
    wh[:, 576:640] = U_main[:, 64:128]
    w46[0:5, 640:704] = W_main[:, 128:192]    # o
    w46[45, 640:704] = b_main[128:192]
    wh[:, 640:704] = U_main[:, 128:192]
    w46[0:5, 704:768] = W_main[:, 192:256]    # cm
    w46[45, 704:768] = b_main[192:256]
    wh[:, 704:768] = U_main[:, 192:256]
    for k in range(K):
        c = 768 + 64 * k
        w46[5 + 5 * k:10 + 5 * k, c:c + 64] = W_aux[k, :, 64:128]
        w46[45, c:c + 64] = b_aux[k, 64:128]
        wh[:, c:c + 64] = U_aux[k, :, 64:128]

    watt = np.asarray(inp["W_att"], f32).T.copy()
    # phase 2 column order [i f o g] (torch gate order is i,f,g,o)
    perm = np.concatenate([np.arange(0, 128), np.arange(192, 256),
                           np.arange(128, 192)])
    wca2 = np.zeros((H + 1, 4 * H), f32)
    wca2[0:H] = np.asarray(inp["W_ih"], f32).T[:, perm]
    wca2[H] = (np.asarray(inp["b_ih"], f32) + np.asarray(inp["b_hh"], f32))[perm]
    wcb = np.asarray(inp["W_hh"], f32).T[:, perm].copy()
    linw = np.asarray(inp["lin_W"], f32).reshape(H, 1)

    bf = ml_dtypes.bfloat16
    return dict(
        w46=w46.astype(bf), wh=wh.astype(bf), watt=watt.astype(bf),
        wca2=wca2.astype(bf), wcb=wcb.astype(bf), linw=linw.astype(bf),
        idf32=np.eye(128, dtype=f32),
    )


def kernel(**inputs) -> np.ndarray:
    n_steps = int(os.environ.get("KERNEL_STEPS", S))
    names = ["Y"] + ["x%d" % i for i in range(1, 9)]
    bf = ml_dtypes.bfloat16
    big = np.empty((n_steps, XR_ROWS, B), np.float32)
    for i, n in enumerate(names):
        a = np.asarray(inputs[n], np.float32)[:n_steps]       # (s, B, F)
        big[:, 5 * i:5 * i + 5, :] = a.transpose(0, 2, 1)
    big[:, 45, :] = 1.0
    big = big.astype(bf)
    wmaps = _prep_weights(inputs)
    b_att = float(np.asarray(inputs["b_att"]).reshape(-1)[0])
    lin_b = float(np.asarray(inputs["lin_b"]).reshape(-1)[0])

    nc = _build(n_steps, b_att)
    ones = np.ones((1, 8 * BL), bf)
    in_maps = []
    for c in range(NC):
        m = dict(wmaps)
        m["xin"] = np.ascontiguousarray(big[:, :, c * BL:(c + 1) * BL])
        m["onesrow"] = ones
        in_maps.append(m)

    trace = bool(int(os.environ.get("KERNEL_TRACE", "0")))
    res = run_bass_kernel_spmd(nc, in_maps, core_ids=list(range(NC)),
                               trace=trace)
    LAST_RESULTS["exec_time_ns"] = res.exec_time_ns
    LAST_RESULTS["trace"] = res.instructions_and_trace

    outs = [r["out"] for r in res.results]  # each (n_steps, BL, 1)
    full = np.concatenate(outs, axis=1) + lin_b
    return full.astype(np.float32)
